# revision 3
# baseline (speedup 1.0000x reference)
"""Trainium2 Bass kernel for CompositeRGCN (8 NeuronCores).

Math (per sample): only node 0's output is consumed downstream, so the whole
R-GCN layer collapses to a row-0-coefficient form.  With

  deg[r, i] = 1 + #edges(et=r, dst=i)
  norm_e    = rsqrt(deg[et_e, src_e] * deg[et_e, dst_e])
  C[r, n]   = sum_{e: dst_e=0, et_e=r, src_e=n} norm_e + (n==0)/deg[r, 0]
  C[R, 0]   = 1                      (self-connection x @ W0 as an extra "relation")

the node-0 hidden state is
  h0 = leaky_relu( sum_{r,n} C[r, n] * (x[n] @ Wcat[r]), 0.1 ),  Wcat = [W_rel; W0]
and the outputs are log_softmax(h0 @ Wg + bg), log_softmax(h0 @ Ws + bs).

Distribution: data-parallel over batch for the graph part (64 samples/core),
AllGather of h0, then tensor-parallel over the 50k/25k output columns with an
AllReduce of the per-row exp-sums for log_softmax.

C (pure index preprocessing, ~0.4 MB) is built on host; all FLOPs (the
C@x contraction, the 2100x300 projection, the 512x300x75000 output matmuls,
exp/log-softmax) run on the NeuronCores.
"""

import os
import sys

sys.path.insert(0, "/opt/trn_rl_repo")

import numpy as np

import concourse.bass as bass  # noqa: F401  (bass must import before mybir use)
import concourse.mybir as mybir
import concourse.tile as tile
from concourse import bacc, bass_utils

# Problem shapes (hardcoded per contract).
B, N, D, R, E = 512, 32, 300, 6, 256
NG, NS = 50000, 25000
NCORES = 8
BPC = B // NCORES           # 64 samples per core
NGRP = BPC // 4             # 16 groups of 4 samples (4*32 = 128 = K)
RX = R + 1                  # 7 "relations" including W0
GRP_COLS = RX * 4           # 28 columns per group in the block-diag C
KH = D + 1                  # 301 contraction rows for the head matmul (bias row)
GW, SW = NG // NCORES, NS // NCORES   # per-core head widths: 6250 / 3125

# d-dimension chunking (300 = 128 + 128 + 44; with bias row: 128 + 128 + 45)
DCH = [(0, 128), (128, 128), (256, 44)]
JCH = [(0, 128), (128, 128), (256, 45)]   # rows of [h; 1] / [W; bias]
NKC = len(DCH) * RX                       # 21 K-chunks of the 2100-dim contraction


def _col_tiles():
    tiles = []
    for head, base, w in ((0, 0, GW), (1, GW, SW)):
        st = 0
        while st < w:
            csz = min(512, w - st)
            tiles.append((head, base + st, csz))
            st += csz
    return tiles


COLTILES = _col_tiles()          # 13 G tiles + 7 S tiles
NGT = sum(1 for t in COLTILES if t[0] == 0)
NCT = len(COLTILES)

_CACHE = {}
LAST_RESULTS = None  # BassKernelResults of the most recent run (for test harness)


def build_nc():
    nc = bacc.Bacc("TRN2", target_bir_lowering=False, debug=False, num_devices=NCORES)
    f32, f32r, bf16 = mybir.dt.float32, mybir.dt.float32r, mybir.dt.bfloat16

    xg_d = nc.dram_tensor("xg", [128, NGRP * D], f32, kind="ExternalInput")
    cblk_d = nc.dram_tensor("cblk", [128, NGRP * GRP_COLS], f32, kind="ExternalInput")
    wcat_d = nc.dram_tensor("wcat", [128, NKC * D], f32, kind="ExternalInput")
    wc_d = nc.dram_tensor("wc", [3, 128, GW + SW], f32, kind="ExternalInput")
    outg_d = nc.dram_tensor("outg", [B, GW], f32, kind="ExternalOutput")
    outs_d = nc.dram_tensor("outs", [B, SW], f32, kind="ExternalOutput")

    with tile.TileContext(nc) as tc:
        with (
            tc.tile_pool(name="io", bufs=1) as io,
            tc.tile_pool(name="gn", bufs=1) as gn,
            tc.tile_pool(name="wt", bufs=6) as wt,
            tc.tile_pool(name="ze", bufs=1) as ze,
            tc.tile_pool(name="st", bufs=1) as st,
            tc.tile_pool(name="po", bufs=4) as po,
            tc.tile_pool(name="pg1", bufs=2, space="PSUM") as pg1,
            tc.tile_pool(name="pg2", bufs=3, space="PSUM") as pg2,
            tc.tile_pool(name="pz", bufs=3, space="PSUM") as pz,
            tc.tile_pool(name="dram", bufs=1, space="DRAM") as dram,
        ):
            # ---------------- Phase G: per-sample graph contraction ----------
            xg_t = io.tile([128, NGRP * D], f32)
            cblk_t = io.tile([128, NGRP * GRP_COLS], f32)
            wcat_t = io.tile([128, NKC * D], f32)
            nc.sync.dma_start(out=xg_t[:], in_=xg_d.ap())
            nc.sync.dma_start(out=cblk_t[:], in_=cblk_d.ap())
            nc.sync.dma_start(out=wcat_t[:], in_=wcat_d.ap())

            # U[(s,r) chunk][d, sample] = sum_n C[b,r,n] x[b,n,d]
            ut_t = gn.tile([128, NKC * BPC], f32)
            ut_v = ut_t.rearrange("p (k e) -> p k e", k=NKC)
            for g in range(NGRP):
                for si, (d0, dsz) in enumerate(DCH):
                    pmm = pg1.tile([128, GRP_COLS], f32, tag="g1")
                    nc.tensor.matmul(
                        pmm[0:dsz, :],
                        lhsT=xg_t[:, g * D + d0 : g * D + d0 + dsz],
                        rhs=cblk_t[:, g * GRP_COLS : (g + 1) * GRP_COLS],
                        start=True,
                        stop=True,
                    )
                    # psum [dsz, (r,b)] -> ut chunks (si*RX + r), columns g*4+b
                    src = pmm[0:dsz, :].rearrange("p (r e) -> p r e", r=RX)
                    dst = ut_v[0:dsz, si * RX : (si + 1) * RX, g * 4 : g * 4 + 4]
                    nc.vector.tensor_copy(dst, src)

            # X0preT = Wcat^T-contract over the 21 chunks; leaky_relu -> h0T
            h0_t = gn.tile([128, 3 * BPC], f32)
            nc.vector.memset(h0_t[:], 0.0)
            for mi, (m0, msz) in enumerate(DCH):
                pacc = pg2.tile([128, BPC], f32, tag="g2")
                for k in range(NKC):
                    ksz = DCH[k // RX][1]
                    nc.tensor.matmul(
                        pacc[0:msz, :],
                        lhsT=wcat_t[0:ksz, k * D + m0 : k * D + m0 + msz],
                        rhs=ut_v[0:ksz, k, :],
                        start=(k == 0),
                        stop=(k == NKC - 1),
                    )
                lk_t = gn.tile([128, BPC], f32, tag="lk", bufs=2)
                nc.scalar.mul(lk_t[0:msz, :], pacc[0:msz, :], 0.1)
                nc.vector.tensor_max(
                    h0_t[0:msz, mi * BPC : (mi + 1) * BPC],
                    pacc[0:msz, :],
                    lk_t[0:msz, :],
                )
            # bias row (K index 300) = 1.0, lives at chunk 2, partition 44.
            # Engines can't start at partition 44; DMA can.
            ones_t = gn.tile([1, BPC], f32)
            nc.vector.memset(ones_t[:], 1.0)
            nc.sync.dma_start(out=h0_t[44:45, 2 * BPC : 3 * BPC], in_=ones_t[0:1, :])

            # ---------------- AllGather h0 across the 8 cores ----------------
            cc_h0i = dram.tile([128, 3 * BPC], f32)
            cc_h0o = dram.tile([NCORES, 128, 3 * BPC], f32, addr_space="Shared")
            nc.sync.dma_start(out=cc_h0i[:], in_=h0_t[:])
            nc.gpsimd.collective_compute(
                "AllGather",
                mybir.AluOpType.bypass,
                replica_groups=[list(range(NCORES))],
                ins=[cc_h0i.opt()],
                outs=[cc_h0o.opt()],
            )
            hf_t = st.tile([128, 3 * B], f32)
            hr_t = st.tile([128, 3 * B], f32r)
            for j, (j0, jsz) in enumerate(JCH):
                # [core][dd][j*64+b] -> [dd][j*512 + core*64 + b]
                nc.sync.dma_start(
                    out=hf_t[0:jsz, j * B : (j + 1) * B].rearrange(
                        "p (c e) -> p c e", c=NCORES
                    ),
                    in_=cc_h0o[:, 0:jsz, j * BPC : (j + 1) * BPC].rearrange(
                        "c p e -> p c e"
                    ),
                )
                nc.vector.tensor_copy(
                    hr_t[0:jsz, j * B : (j + 1) * B], hf_t[0:jsz, j * B : (j + 1) * B]
                )

            # ---------------- Phase M: big matmul + exp/sums -----------------
            sums_t = st.tile([128, 4 * NCT], f32)
            z_tiles = {}
            for ci, (head, c0, csz) in enumerate(COLTILES):
                # fp32r is only legal/faster for even, >=256-wide moving dims;
                # tail tiles (106/53 cols) run in plain fp32.
                use_r = csz >= 256
                wr = []
                for j, (j0, jsz) in enumerate(JCH):
                    wf_t = wt.tile([128, 512], f32, tag="wf")
                    nc.sync.dma_start(
                        out=wf_t[0:jsz, 0:csz], in_=wc_d[j, 0:jsz, c0 : c0 + csz]
                    )
                    if use_r:
                        wr_t = wt.tile([128, 512], f32r, tag="wr")
                        nc.vector.tensor_copy(wr_t[0:jsz, 0:csz], wf_t[0:jsz, 0:csz])
                        wr.append(wr_t)
                    else:
                        wr.append(wf_t)
                lhs_t = hr_t if use_r else hf_t
                for m in range(4):
                    pzt = pz.tile([128, 512], f32, tag="z")
                    for j, (j0, jsz) in enumerate(JCH):
                        nc.tensor.matmul(
                            pzt[:, 0:csz],
                            lhsT=lhs_t[0:jsz, j * B + m * 128 : j * B + (m + 1) * 128],
                            rhs=wr[j][0:jsz, 0:csz],
                            start=(j == 0),
                            stop=(j == 2),
                        )
                    e_t = ze.tile([128, csz], bf16, tag=f"z{ci}_{m}", name=f"z{ci}_{m}")
                    nc.scalar.activation(
                        e_t[:],
                        pzt[:, 0:csz],
                        mybir.ActivationFunctionType.Exp,
                        accum_out=sums_t[:, m * NCT + ci : m * NCT + ci + 1],
                    )
                    z_tiles[(ci, m)] = e_t

            # per-head row sums -> AllReduce -> reciprocals
            s2_t = st.tile([128, 8], f32)
            for m in range(4):
                nc.vector.tensor_reduce(
                    s2_t[:, m : m + 1],
                    sums_t[:, m * NCT : m * NCT + NGT],
                    axis=mybir.AxisListType.X,
                    op=mybir.AluOpType.add,
                )
                nc.vector.tensor_reduce(
                    s2_t[:, 4 + m : 5 + m],
                    sums_t[:, m * NCT + NGT : (m + 1) * NCT],
                    axis=mybir.AxisListType.X,
                    op=mybir.AluOpType.add,
                )
            cc_si = dram.tile([128, 8], f32)
            cc_so = dram.tile([128, 8], f32, addr_space="Shared")
            nc.sync.dma_start(out=cc_si[:], in_=s2_t[:])
            nc.gpsimd.collective_compute(
                "AllReduce",
                mybir.AluOpType.add,
                replica_groups=[list(range(NCORES))],
                ins=[cc_si.opt()],
                outs=[cc_so.opt()],
            )
            s2g_t = st.tile([128, 8], f32)
            nc.sync.dma_start(out=s2g_t[:], in_=cc_so[:])
            rcp_t = st.tile([128, 8], f32)
            nc.vector.reciprocal(rcp_t[:], s2g_t[:])

            # ---------------- pass 2: out = ln(exp(z) / S) -------------------
            for ci, (head, c0, csz) in enumerate(COLTILES):
                for m in range(4):
                    o_t = po.tile([128, 512], f32, tag="po")
                    nc.scalar.activation(
                        o_t[:, 0:csz],
                        z_tiles[(ci, m)][:],
                        mybir.ActivationFunctionType.Ln,
                        scale=rcp_t[:, head * 4 + m : head * 4 + m + 1],
                    )
                    dst = outg_d if head == 0 else outs_d
                    lc0 = c0 if head == 0 else c0 - GW
                    nc.sync.dma_start(
                        out=dst[m * 128 : (m + 1) * 128, lc0 : lc0 + csz],
                        in_=o_t[:, 0:csz],
                    )

    nc.compile()
    return nc


def _prep(x, edge_index, edge_type, W_rel, W0, Wg, bg, Ws, bs):
    """Host-side index preprocessing + per-core input shards."""
    x = np.asarray(x, dtype=np.float32)
    ei = np.asarray(edge_index).astype(np.int64)
    et = np.asarray(edge_type).astype(np.int64)
    W_rel = np.asarray(W_rel, dtype=np.float32)
    W0 = np.asarray(W0, dtype=np.float32)
    Wg = np.asarray(Wg, dtype=np.float32)
    bg = np.asarray(bg, dtype=np.float32)
    Ws = np.asarray(Ws, dtype=np.float32)
    bs = np.asarray(bs, dtype=np.float32)

    src, dst = ei[:, 0, :], ei[:, 1, :]
    bidx = np.arange(B)[:, None]
    deg = np.ones((B, R, N), np.float32)
    np.add.at(deg, (bidx, et, dst), np.float32(1.0))
    norm = 1.0 / np.sqrt(deg[bidx, et, src] * deg[bidx, et, dst])  # [B, E]
    C = np.zeros((B, RX, N), np.float32)
    np.add.at(C, (bidx, et, src), np.where(dst == 0, norm, 0.0).astype(np.float32))
    C[:, :R, 0] += 1.0 / deg[:, :, 0]
    C[:, R, 0] += 1.0

    # wcat: [dd, k=(s,r), m] with rows (r, s*128+dd) of [W_rel; W0]
    wfull = np.concatenate([W_rel, W0[None]], axis=0)  # [7, 300, 300]
    wcat = np.zeros((128, NKC, D), np.float32)
    for si, (d0, dsz) in enumerate(DCH):
        for r in range(RX):
            wcat[0:dsz, si * RX + r, :] = wfull[r, d0 : d0 + dsz, :]
    wcat = wcat.reshape(128, NKC * D)

    in_maps = []
    for c in range(NCORES):
        sl = slice(c * BPC, (c + 1) * BPC)
        # xg[(b4, n), g*300+d]
        xg = (
            x[sl]
            .reshape(NGRP, 4, N, D)
            .transpose(1, 2, 0, 3)
            .reshape(128, NGRP * D)
            .copy()
        )
        # cblk[(b4, n), g*28 + r*4 + bcol] = C[c*64+g*4+bcol, r, n] * (b4==bcol)
        Cc = C[sl].reshape(NGRP, 4, RX, N)
        cb = np.zeros((NGRP, 4, N, RX, 4), np.float32)
        for b in range(4):
            cb[:, b, :, :, b] = Cc[:, b].transpose(0, 2, 1)
        cblk = cb.reshape(NGRP, 128, GRP_COLS).transpose(1, 0, 2).reshape(
            128, NGRP * GRP_COLS
        ).copy()
        # wc: [j, dd, col]: rows = [W; bias] K-chunks; cols = G slice ++ S slice
        wgs = np.concatenate(
            [Wg[:, c * GW : (c + 1) * GW], Ws[:, c * SW : (c + 1) * SW]], axis=1
        )
        bias = np.concatenate([bg[c * GW : (c + 1) * GW], bs[c * SW : (c + 1) * SW]])
        wc = np.zeros((3, 128, GW + SW), np.float32)
        wc[0] = wgs[0:128]
        wc[1] = wgs[128:256]
        wc[2, 0:44] = wgs[256:300]
        wc[2, 44] = bias
        in_maps.append({"xg": xg, "cblk": cblk, "wcat": wcat, "wc": wc})
    return in_maps


def kernel(x, edge_index, edge_type, W_rel, W0, Wg, bg, Ws, bs):
    global LAST_RESULTS
    if "nc" not in _CACHE:
        _CACHE["nc"] = build_nc()
    nc = _CACHE["nc"]
    in_maps = _prep(x, edge_index, edge_type, W_rel, W0, Wg, bg, Ws, bs)
    trace = bool(int(os.environ.get("KERNEL_TRACE", "0")))
    res = bass_utils.run_bass_kernel_spmd(
        nc, in_maps, core_ids=list(range(NCORES)), trace=trace
    )
    LAST_RESULTS = res
    outg = np.concatenate([res.results[c]["outg"] for c in range(NCORES)], axis=1)
    outs = np.concatenate([res.results[c]["outs"] for c in range(NCORES)], axis=1)
    return outg, outs


# revision 4
# speedup vs baseline: 1.1566x; 1.1566x over previous
"""Trainium2 Bass kernel for CompositeRGCN (8 NeuronCores).

Math (per sample): only node 0's output is consumed downstream, so the whole
R-GCN layer collapses to a row-0-coefficient form.  With

  deg[r, i] = 1 + #edges(et=r, dst=i)
  norm_e    = rsqrt(deg[et_e, src_e] * deg[et_e, dst_e])
  C[r, n]   = sum_{e: dst_e=0, et_e=r, src_e=n} norm_e + (n==0)/deg[r, 0]
  C[R, 0]   = 1                      (self-connection x @ W0 as an extra "relation")

the node-0 hidden state is
  h0 = leaky_relu( sum_{r,n} C[r, n] * (x[n] @ Wcat[r]), 0.1 ),  Wcat = [W_rel; W0]
and the outputs are log_softmax(h0 @ Wg + bg), log_softmax(h0 @ Ws + bs).

Distribution: data-parallel over batch for the graph part (64 samples/core),
AllGather of h0, then tensor-parallel over the 50k/25k output columns with an
AllReduce of the per-row exp-sums for log_softmax.

Kernel structure (v2): all output-projection weights are SBUF-resident and
prefetched during the collectives' entry barrier; softmax pass 2 recomputes
the logits from the resident weights (PE is idle there) and subtracts log(S)
on the vector engine, so logits never round-trip through a low-precision
store.  The S head's stats AllReduce is issued early so its pass 2 overlaps
the G head's pass 1.

C (pure index preprocessing, ~0.4 MB) is built on host; all FLOPs (the
C@x contraction, the 2100x300 projection, the 512x300x75000 output matmuls,
exp/log-softmax) run on the NeuronCores.
"""

import os
import sys

sys.path.insert(0, "/opt/trn_rl_repo")

import numpy as np

import concourse.bass as bass  # noqa: F401
import concourse.mybir as mybir
import concourse.tile as tile
from concourse import bacc, bass_utils

# Problem shapes (hardcoded per contract).
B, N, D, R, E = 512, 32, 300, 6, 256
NG, NS = 50000, 25000
NCORES = 8
BPC = B // NCORES           # 64 samples per core
NGRP = BPC // 4             # 16 groups of 4 samples (4*32 = 128 = K)
RX = R + 1                  # 7 "relations" including W0
GRP_COLS = RX * 4           # 28 columns per group in the block-diag C
KH = D + 1                  # 301 contraction rows for the head matmul (bias row)
GW, SW = NG // NCORES, NS // NCORES   # per-core head widths: 6250 / 3125

DCH = [(0, 128), (128, 128), (256, 44)]
JCH = [(0, 128), (128, 128), (256, 45)]   # rows of [h; 1] / [W; bias]
NKC = len(DCH) * RX                       # 21 K-chunks of the 2100-dim contraction


def _col_tiles():
    tiles = []
    for head, base, w in ((0, 0, GW), (1, GW, SW)):
        st = 0
        while st < w:
            csz = min(512, w - st)
            tiles.append((head, base + st, csz))
            st += csz
    return tiles


COLTILES = _col_tiles()          # 13 G tiles + 7 S tiles
NGT = sum(1 for t in COLTILES if t[0] == 0)
NCT = len(COLTILES)
S_CIS = [ci for ci, t in enumerate(COLTILES) if t[0] == 1]
G_CIS = [ci for ci, t in enumerate(COLTILES) if t[0] == 0]

_CACHE = {}
LAST_RESULTS = None


def build_nc():
    nc = bacc.Bacc("TRN2", target_bir_lowering=False, debug=False, num_devices=NCORES)
    f32, f32r, bf16 = mybir.dt.float32, mybir.dt.float32r, mybir.dt.bfloat16

    xg_d = nc.dram_tensor("xg", [128, NGRP * D], f32, kind="ExternalInput")
    cblk_d = nc.dram_tensor("cblk", [128, NGRP * GRP_COLS], f32, kind="ExternalInput")
    wcat_d = nc.dram_tensor("wcat", [128, NKC * D], f32, kind="ExternalInput")
    # output-projection weights, pre-chunked; tagged float32r so the PE can
    # stream them at full rate with no on-chip cast (f32r = hw-rounded f32)
    wc_d = nc.dram_tensor("wc", [3, 128, GW + SW], f32r, kind="ExternalInput")
    outg_d = nc.dram_tensor("outg", [B, GW], f32, kind="ExternalOutput")
    outs_d = nc.dram_tensor("outs", [B, SW], f32, kind="ExternalOutput")

    with tile.TileContext(nc) as tc:
        with (
            tc.tile_pool(name="io", bufs=1) as io,
            tc.tile_pool(name="gn", bufs=1) as gn,
            tc.tile_pool(name="wres", bufs=1) as wres,
            tc.tile_pool(name="st", bufs=1) as st,
            tc.tile_pool(name="po", bufs=3) as po,
            tc.tile_pool(name="pg1", bufs=3, space="PSUM") as pg1,
            tc.tile_pool(name="pg2", bufs=3, space="PSUM") as pg2,
            tc.tile_pool(name="pz", bufs=2, space="PSUM") as pz,
            tc.tile_pool(name="dram", bufs=1, space="DRAM") as dram,
        ):
            # -------- resident output-projection weights (prefetch at t=0) ---
            wtiles = {}
            for ci, (head, c0, csz) in enumerate(COLTILES):
                for j, (j0, jsz) in enumerate(JCH):
                    w_t = wres.tile(
                        [128, csz], f32r, tag=f"w{ci}_{j}", name=f"w{ci}_{j}"
                    )
                    nc.sync.dma_start(
                        out=w_t[0:jsz, :], in_=wc_d[j, 0:jsz, c0 : c0 + csz]
                    )
                    wtiles[(ci, j)] = w_t

            # ---------------- Phase G: per-sample graph contraction ----------
            xg_t = io.tile([128, NGRP * D], f32)
            cblk_t = io.tile([128, NGRP * GRP_COLS], f32)
            wcat_t = io.tile([128, NKC * D], f32)
            nc.sync.dma_start(out=xg_t[:], in_=xg_d.ap())
            nc.sync.dma_start(out=cblk_t[:], in_=cblk_d.ap())
            nc.sync.dma_start(out=wcat_t[:], in_=wcat_d.ap())

            ut_t = gn.tile([128, NKC * BPC], f32)
            ut_v = ut_t.rearrange("p (k e) -> p k e", k=NKC)
            for g in range(NGRP):
                for si, (d0, dsz) in enumerate(DCH):
                    pmm = pg1.tile([128, GRP_COLS], f32, tag="g1")
                    nc.tensor.matmul(
                        pmm[0:dsz, :],
                        lhsT=xg_t[:, g * D + d0 : g * D + d0 + dsz],
                        rhs=cblk_t[:, g * GRP_COLS : (g + 1) * GRP_COLS],
                        start=True,
                        stop=True,
                    )
                    src = pmm[0:dsz, :].rearrange("p (r e) -> p r e", r=RX)
                    dst = ut_v[0:dsz, si * RX : (si + 1) * RX, g * 4 : g * 4 + 4]
                    eng = nc.vector if (g % 2 == 0) else nc.scalar
                    if eng is nc.vector:
                        nc.vector.tensor_copy(dst, src)
                    else:
                        nc.scalar.copy(dst, src)

            h0_t = gn.tile([128, 3 * BPC], f32)
            nc.vector.memset(h0_t[:], 0.0)
            for mi, (m0, msz) in enumerate(DCH):
                pacc = pg2.tile([128, BPC], f32, tag="g2")
                for k in range(NKC):
                    ksz = DCH[k // RX][1]
                    nc.tensor.matmul(
                        pacc[0:msz, :],
                        lhsT=wcat_t[0:ksz, k * D + m0 : k * D + m0 + msz],
                        rhs=ut_v[0:ksz, k, :],
                        start=(k == 0),
                        stop=(k == NKC - 1),
                    )
                lk_t = gn.tile([128, BPC], f32, tag="lk", bufs=2)
                nc.scalar.mul(lk_t[0:msz, :], pacc[0:msz, :], 0.1)
                nc.vector.tensor_max(
                    h0_t[0:msz, mi * BPC : (mi + 1) * BPC],
                    pacc[0:msz, :],
                    lk_t[0:msz, :],
                )
            # bias row (K index 300) = 1.0 at chunk 2, partition 44 (DMA: engines
            # can't start at partition 44)
            ones_t = gn.tile([1, BPC], f32)
            nc.vector.memset(ones_t[:], 1.0)
            nc.sync.dma_start(out=h0_t[44:45, 2 * BPC : 3 * BPC], in_=ones_t[0:1, :])

            # ---------------- AllGather h0 across the 8 cores ----------------
            cc_h0i = dram.tile([128, 3 * BPC], f32)
            cc_h0o = dram.tile([NCORES, 128, 3 * BPC], f32, addr_space="Shared")
            nc.sync.dma_start(out=cc_h0i[:], in_=h0_t[:])
            nc.gpsimd.collective_compute(
                "AllGather",
                mybir.AluOpType.bypass,
                replica_groups=[list(range(NCORES))],
                ins=[cc_h0i.opt()],
                outs=[cc_h0o.opt()],
            )
            hf_t = st.tile([128, 3 * B], f32)
            hr_t = st.tile([128, 3 * B], f32r)
            for j, (j0, jsz) in enumerate(JCH):
                nc.sync.dma_start(
                    out=hf_t[0:jsz, j * B : (j + 1) * B].rearrange(
                        "p (c e) -> p c e", c=NCORES
                    ),
                    in_=cc_h0o[:, 0:jsz, j * BPC : (j + 1) * BPC].rearrange(
                        "c p e -> p c e"
                    ),
                )
                nc.vector.tensor_copy(
                    hr_t[0:jsz, j * B : (j + 1) * B], hf_t[0:jsz, j * B : (j + 1) * B]
                )

            # ---------------- pass 1: matmul + exp row-sums ------------------
            sums_t = st.tile([128, 4 * NCT], f32)

            def mm_tile(ci, m, ppool):
                head, c0, csz = COLTILES[ci]
                use_r = csz >= 256
                pzt = ppool.tile([128, 512], f32, tag="z")
                for j, (j0, jsz) in enumerate(JCH):
                    rhs = wtiles[(ci, j)][0:jsz, :]
                    lhs = (hr_t if use_r else hf_t)[
                        0:jsz, j * B + m * 128 : j * B + (m + 1) * 128
                    ]
                    if not use_r:
                        rhs = rhs.bitcast(mybir.dt.float32)
                    nc.tensor.matmul(
                        pzt[:, 0:csz], lhsT=lhs, rhs=rhs,
                        start=(j == 0), stop=(j == 2),
                    )
                return pzt

            esc_pool = st
            for ci in S_CIS + G_CIS:
                head, c0, csz = COLTILES[ci]
                for m in range(4):
                    pzt = mm_tile(ci, m, pz)
                    e_t = esc_pool.tile([128, 512], mybir.dt.bfloat16, tag="esc", bufs=2)
                    nc.scalar.activation(
                        e_t[:, 0:csz],
                        pzt[:, 0:csz],
                        mybir.ActivationFunctionType.Exp,
                        accum_out=sums_t[:, m * NCT + ci : m * NCT + ci + 1],
                    )

                if ci == S_CIS[-1] or ci == G_CIS[-1]:
                    # this head's pass-1 done: reduce + AllReduce its sums
                    hd = COLTILES[ci][0]
                    cis = S_CIS if hd == 1 else G_CIS
                    s2_t = st.tile([128, 4], f32, tag=f"s2_{hd}", name=f"s2_{hd}")
                    for m in range(4):
                        lo = m * NCT + cis[0]
                        nc.vector.tensor_reduce(
                            s2_t[:, m : m + 1],
                            sums_t[:, lo : lo + len(cis)],
                            axis=mybir.AxisListType.X,
                            op=mybir.AluOpType.add,
                        )
                    cc_i = dram.tile([128, 4], f32, name=f"cci{hd}")
                    cc_o = dram.tile([128, 4], f32, addr_space="Shared", name=f"cco{hd}")
                    nc.sync.dma_start(out=cc_i[:], in_=s2_t[:])
                    nc.gpsimd.collective_compute(
                        "AllReduce",
                        mybir.AluOpType.add,
                        replica_groups=[list(range(NCORES))],
                        ins=[cc_i.opt()],
                        outs=[cc_o.opt()],
                    )
                    sg_t = st.tile([128, 4], f32, tag=f"sg_{hd}", name=f"sg_{hd}")
                    nc.sync.dma_start(out=sg_t[:], in_=cc_o[:])
                    logs_t = st.tile([128, 4], f32, tag=f"lg_{hd}", name=f"lg_{hd}")
                    nc.scalar.activation(
                        logs_t[:], sg_t[:], mybir.ActivationFunctionType.Ln
                    )
                    if hd == 1:
                        logs_s = logs_t
                    else:
                        logs_g = logs_t

            # ---------------- pass 2: recompute z, subtract log(S) ----------
            for ci in S_CIS + G_CIS:
                head, c0, csz = COLTILES[ci]
                logs = logs_s if head == 1 else logs_g
                for m in range(4):
                    pzt = mm_tile(ci, m, pz)
                    o_t = po.tile([128, 512], f32, tag="po")
                    nc.vector.tensor_scalar(
                        o_t[:, 0:csz],
                        pzt[:, 0:csz],
                        logs[:, m : m + 1],
                        None,
                        op0=mybir.AluOpType.subtract,
                    )
                    dst = outg_d if head == 0 else outs_d
                    lc0 = c0 if head == 0 else c0 - GW
                    nc.sync.dma_start(
                        out=dst[m * 128 : (m + 1) * 128, lc0 : lc0 + csz],
                        in_=o_t[:, 0:csz],
                    )

    nc.compile()
    return nc


def _prep(x, edge_index, edge_type, W_rel, W0, Wg, bg, Ws, bs):
    """Host-side index preprocessing + per-core input shards."""
    x = np.asarray(x, dtype=np.float32)
    ei = np.asarray(edge_index).astype(np.int64)
    et = np.asarray(edge_type).astype(np.int64)
    W_rel = np.asarray(W_rel, dtype=np.float32)
    W0 = np.asarray(W0, dtype=np.float32)
    Wg = np.asarray(Wg, dtype=np.float32)
    bg = np.asarray(bg, dtype=np.float32)
    Ws = np.asarray(Ws, dtype=np.float32)
    bs = np.asarray(bs, dtype=np.float32)

    src, dst = ei[:, 0, :], ei[:, 1, :]
    bidx = np.arange(B)[:, None]
    deg = np.ones((B, R, N), np.float32)
    np.add.at(deg, (bidx, et, dst), np.float32(1.0))
    norm = 1.0 / np.sqrt(deg[bidx, et, src] * deg[bidx, et, dst])  # [B, E]
    C = np.zeros((B, RX, N), np.float32)
    np.add.at(C, (bidx, et, src), np.where(dst == 0, norm, 0.0).astype(np.float32))
    C[:, :R, 0] += 1.0 / deg[:, :, 0]
    C[:, R, 0] += 1.0

    wfull = np.concatenate([W_rel, W0[None]], axis=0)  # [7, 300, 300]
    wcat = np.zeros((128, NKC, D), np.float32)
    for si, (d0, dsz) in enumerate(DCH):
        for r in range(RX):
            wcat[0:dsz, si * RX + r, :] = wfull[r, d0 : d0 + dsz, :]
    wcat = wcat.reshape(128, NKC * D)

    in_maps = []
    for c in range(NCORES):
        sl = slice(c * BPC, (c + 1) * BPC)
        xg = (
            x[sl]
            .reshape(NGRP, 4, N, D)
            .transpose(1, 2, 0, 3)
            .reshape(128, NGRP * D)
            .copy()
        )
        Cc = C[sl].reshape(NGRP, 4, RX, N)
        cb = np.zeros((NGRP, 4, N, RX, 4), np.float32)
        for b in range(4):
            cb[:, b, :, :, b] = Cc[:, b].transpose(0, 2, 1)
        cblk = cb.reshape(NGRP, 128, GRP_COLS).transpose(1, 0, 2).reshape(
            128, NGRP * GRP_COLS
        ).copy()
        wgs = np.concatenate(
            [Wg[:, c * GW : (c + 1) * GW], Ws[:, c * SW : (c + 1) * SW]], axis=1
        )
        bias = np.concatenate([bg[c * GW : (c + 1) * GW], bs[c * SW : (c + 1) * SW]])
        wc = np.zeros((3, 128, GW + SW), np.float32)
        wc[0] = wgs[0:128]
        wc[1] = wgs[128:256]
        wc[2, 0:44] = wgs[256:300]
        wc[2, 44] = bias
        in_maps.append({"xg": xg, "cblk": cblk, "wcat": wcat, "wc": wc})
    return in_maps


def kernel(x, edge_index, edge_type, W_rel, W0, Wg, bg, Ws, bs):
    global LAST_RESULTS
    if "nc" not in _CACHE:
        _CACHE["nc"] = build_nc()
    nc = _CACHE["nc"]
    in_maps = _prep(x, edge_index, edge_type, W_rel, W0, Wg, bg, Ws, bs)
    trace = bool(int(os.environ.get("KERNEL_TRACE", "0")))
    res = bass_utils.run_bass_kernel_spmd(
        nc, in_maps, core_ids=list(range(NCORES)), trace=trace
    )
    LAST_RESULTS = res
    outg = np.concatenate([res.results[c]["outg"] for c in range(NCORES)], axis=1)
    outs = np.concatenate([res.results[c]["outs"] for c in range(NCORES)], axis=1)
    return outg, outs


# revision 6
# speedup vs baseline: 1.2764x; 1.1035x over previous
"""Trainium2 Bass kernel for CompositeRGCN (8 NeuronCores).

Math (per sample): only node 0's output is consumed downstream, so the whole
R-GCN layer collapses to a row-0-coefficient form.  With

  deg[r, i] = 1 + #edges(et=r, dst=i)
  norm_e    = rsqrt(deg[et_e, src_e] * deg[et_e, dst_e])
  C[r, n]   = sum_{e: dst_e=0, et_e=r, src_e=n} norm_e + (n==0)/deg[r, 0]
  C[R, 0]   = 1                      (self-connection x @ W0 as an extra "relation")

the node-0 hidden state is
  h0 = leaky_relu( sum_{r,n} C[r, n] * (x[n] @ Wcat[r]), 0.1 ),  Wcat = [W_rel; W0]
and the outputs are log_softmax(h0 @ Wg + bg), log_softmax(h0 @ Ws + bs).

Distribution: data-parallel over batch for the graph part (64 samples/core),
AllGather of h0, then tensor-parallel over the 50k/25k output columns with an
AllReduce of the per-row exp-sums for log_softmax.

Kernel structure (v2): all output-projection weights are SBUF-resident and
prefetched during the collectives' entry barrier; softmax pass 2 recomputes
the logits from the resident weights (PE is idle there) and subtracts log(S)
on the vector engine, so logits never round-trip through a low-precision
store.  The S head's stats AllReduce is issued early so its pass 2 overlaps
the G head's pass 1.

C (pure index preprocessing, ~0.4 MB) is built on host; all FLOPs (the
C@x contraction, the 2100x300 projection, the 512x300x75000 output matmuls,
exp/log-softmax) run on the NeuronCores.
"""

import os
import sys

sys.path.insert(0, "/opt/trn_rl_repo")

import numpy as np

import concourse.bass as bass  # noqa: F401
import concourse.mybir as mybir
import concourse.tile as tile
from concourse import bacc, bass_utils

# Problem shapes (hardcoded per contract).
B, N, D, R, E = 512, 32, 300, 6, 256
NG, NS = 50000, 25000
NCORES = 8
BPC = B // NCORES           # 64 samples per core
NGRP = BPC // 4             # 16 groups of 4 samples (4*32 = 128 = K)
RX = R + 1                  # 7 "relations" including W0
GRP_COLS = RX * 4           # 28 columns per group in the block-diag C
KH = D + 1                  # 301 contraction rows for the head matmul (bias row)
GW, SW = NG // NCORES, NS // NCORES   # per-core head widths: 6250 / 3125

DCH = [(0, 128), (128, 128), (256, 44)]
JCH = [(0, 128), (128, 128), (256, 45)]   # rows of [h; 1] / [W; bias]
NKC = len(DCH) * RX                       # 21 K-chunks of the 2100-dim contraction


def _col_tiles():
    tiles = []
    for head, base, w in ((0, 0, GW), (1, GW, SW)):
        st = 0
        while st < w:
            csz = min(512, w - st)
            tiles.append((head, base + st, csz))
            st += csz
    return tiles


COLTILES = _col_tiles()          # 13 G tiles + 7 S tiles
NGT = sum(1 for t in COLTILES if t[0] == 0)
NCT = len(COLTILES)
S_CIS = [ci for ci, t in enumerate(COLTILES) if t[0] == 1]
G_CIS = [ci for ci, t in enumerate(COLTILES) if t[0] == 0]

_CACHE = {}
LAST_RESULTS = None


def build_nc():
    nc = bacc.Bacc("TRN2", target_bir_lowering=False, debug=False, num_devices=NCORES)
    f32, f32r, bf16 = mybir.dt.float32, mybir.dt.float32r, mybir.dt.bfloat16

    xg_d = nc.dram_tensor("xg", [128, NGRP * D], f32, kind="ExternalInput")
    cblk_d = nc.dram_tensor("cblk", [128, NGRP * GRP_COLS], f32, kind="ExternalInput")
    wcat_d = nc.dram_tensor("wcat", [128, NKC * D], f32, kind="ExternalInput")
    # output-projection weights, pre-chunked; tagged float32r so the PE can
    # stream them at full rate with no on-chip cast (f32r = hw-rounded f32)
    wc_d = nc.dram_tensor("wc", [3, 128, GW + SW], f32r, kind="ExternalInput")
    outg_d = nc.dram_tensor("outg", [B, GW], f32, kind="ExternalOutput")
    outs_d = nc.dram_tensor("outs", [B, SW], f32, kind="ExternalOutput")

    with tile.TileContext(nc) as tc:
        with (
            tc.tile_pool(name="io", bufs=1) as io,
            tc.tile_pool(name="gn", bufs=1) as gn,
            tc.tile_pool(name="wres", bufs=1) as wres,
            tc.tile_pool(name="st", bufs=1) as st,
            tc.tile_pool(name="po", bufs=3) as po,
            tc.tile_pool(name="pg1", bufs=3, space="PSUM") as pg1,
            tc.tile_pool(name="pg2", bufs=2, space="PSUM") as pg2,
            tc.tile_pool(name="pz", bufs=3, space="PSUM") as pz,
            tc.tile_pool(name="dram", bufs=1, space="DRAM") as dram,
        ):
            # ---------------- Phase G: per-sample graph contraction ----------
            # (input DMAs first so the graph phase starts immediately; the
            # weight prefetch below fills the collectives' entry barrier)
            xg_t = io.tile([128, NGRP * D], f32)
            cblk_t = io.tile([128, NGRP * GRP_COLS], f32)
            wcat_t = io.tile([128, NKC * D], f32)
            nc.sync.dma_start(out=xg_t[:], in_=xg_d.ap())
            nc.sync.dma_start(out=cblk_t[:], in_=cblk_d.ap())
            nc.sync.dma_start(out=wcat_t[:], in_=wcat_d.ap())

            # -------- resident output-projection weights (prefetch) ---------
            wtiles = {}
            for ci, (head, c0, csz) in enumerate(COLTILES):
                for j, (j0, jsz) in enumerate(JCH):
                    w_t = wres.tile(
                        [128, csz], f32r, tag=f"w{ci}_{j}", name=f"w{ci}_{j}"
                    )
                    nc.sync.dma_start(
                        out=w_t[0:jsz, :], in_=wc_d[j, 0:jsz, c0 : c0 + csz]
                    )
                    wtiles[(ci, j)] = w_t

            ut_t = gn.tile([128, NKC * BPC], f32)
            ut_v = ut_t.rearrange("p (k e) -> p k e", k=NKC)
            for g in range(NGRP):
                for si, (d0, dsz) in enumerate(DCH):
                    pmm = pg1.tile([128, GRP_COLS], f32, tag="g1")
                    nc.tensor.matmul(
                        pmm[0:dsz, :],
                        lhsT=xg_t[:, g * D + d0 : g * D + d0 + dsz],
                        rhs=cblk_t[:, g * GRP_COLS : (g + 1) * GRP_COLS],
                        start=True,
                        stop=True,
                    )
                    src = pmm[0:dsz, :].rearrange("p (r e) -> p r e", r=RX)
                    dst = ut_v[0:dsz, si * RX : (si + 1) * RX, g * 4 : g * 4 + 4]
                    eng = nc.vector if (g % 2 == 0) else nc.scalar
                    if eng is nc.vector:
                        nc.vector.tensor_copy(dst, src)
                    else:
                        nc.scalar.copy(dst, src)

            h0_t = gn.tile([128, 3 * BPC], f32)
            nc.vector.memset(h0_t[:], 0.0)
            for mi, (m0, msz) in enumerate(DCH):
                pacc = pg2.tile([128, BPC], f32, tag="g2")
                for k in range(NKC):
                    ksz = DCH[k // RX][1]
                    nc.tensor.matmul(
                        pacc[0:msz, :],
                        lhsT=wcat_t[0:ksz, k * D + m0 : k * D + m0 + msz],
                        rhs=ut_v[0:ksz, k, :],
                        start=(k == 0),
                        stop=(k == NKC - 1),
                    )
                lk_t = gn.tile([128, BPC], f32, tag="lk", bufs=2)
                nc.scalar.mul(lk_t[0:msz, :], pacc[0:msz, :], 0.1)
                nc.vector.tensor_max(
                    h0_t[0:msz, mi * BPC : (mi + 1) * BPC],
                    pacc[0:msz, :],
                    lk_t[0:msz, :],
                )
            # bias row (K index 300) = 1.0 at chunk 2, partition 44 (DMA: engines
            # can't start at partition 44)
            ones_t = gn.tile([1, BPC], f32)
            nc.vector.memset(ones_t[:], 1.0)
            nc.sync.dma_start(out=h0_t[44:45, 2 * BPC : 3 * BPC], in_=ones_t[0:1, :])

            # ---------------- AllGather h0 across the 8 cores ----------------
            cc_h0i = dram.tile([128, 3 * BPC], f32)
            cc_h0o = dram.tile([NCORES, 128, 3 * BPC], f32, addr_space="Shared")
            nc.sync.dma_start(out=cc_h0i[:], in_=h0_t[:])
            nc.gpsimd.collective_compute(
                "AllGather",
                mybir.AluOpType.bypass,
                replica_groups=[list(range(NCORES))],
                ins=[cc_h0i.opt()],
                outs=[cc_h0o.opt()],
            )
            hf_t = st.tile([128, 3 * B], f32)
            hr_t = st.tile([128, 3 * B], f32r)
            for j, (j0, jsz) in enumerate(JCH):
                nc.sync.dma_start(
                    out=hf_t[0:jsz, j * B : (j + 1) * B].rearrange(
                        "p (c e) -> p c e", c=NCORES
                    ),
                    in_=cc_h0o[:, 0:jsz, j * BPC : (j + 1) * BPC].rearrange(
                        "c p e -> p c e"
                    ),
                )
                nc.vector.tensor_copy(
                    hr_t[0:jsz, j * B : (j + 1) * B], hf_t[0:jsz, j * B : (j + 1) * B]
                )

            # PE clock warmup: the HAM un-throttles (1.2 -> 2.4 GHz) only after
            # ~3.4us of dense activity; a short back-to-back f32r burst right
            # after the gather trips it before the real matmul stream begins.
            for i in range(24):
                pw = pz.tile([128, 512], f32, tag="z")
                nc.tensor.matmul(
                    pw[:], lhsT=hr_t[:, 0:128], rhs=hr_t[:, 0:512],
                    start=True, stop=True,
                )

            # ---------------- pass 1: matmul + exp row-sums ------------------
            sums_t = st.tile([128, 4 * NCT], f32)

            def mm_tile(ci, m, ppool):
                head, c0, csz = COLTILES[ci]
                use_r = csz >= 256
                pzt = ppool.tile([128, 512], f32, tag="z")
                for j, (j0, jsz) in enumerate(JCH):
                    rhs = wtiles[(ci, j)][0:jsz, :]
                    lhs = (hr_t if use_r else hf_t)[
                        0:jsz, j * B + m * 128 : j * B + (m + 1) * 128
                    ]
                    if not use_r:
                        rhs = rhs.bitcast(mybir.dt.float32)
                    nc.tensor.matmul(
                        pzt[:, 0:csz], lhsT=lhs, rhs=rhs,
                        start=(j == 0), stop=(j == 2),
                    )
                return pzt

            esc_pool = st
            for ci in S_CIS + G_CIS:
                head, c0, csz = COLTILES[ci]
                for m in range(4):
                    pzt = mm_tile(ci, m, pz)
                    e_t = esc_pool.tile([128, 512], mybir.dt.bfloat16, tag="esc", bufs=2)
                    nc.scalar.activation(
                        e_t[:, 0:csz],
                        pzt[:, 0:csz],
                        mybir.ActivationFunctionType.Exp,
                        accum_out=sums_t[:, m * NCT + ci : m * NCT + ci + 1],
                    )

                if ci == S_CIS[-1] or ci == G_CIS[-1]:
                    # this head's pass-1 done: reduce + AllReduce its sums
                    hd = COLTILES[ci][0]
                    cis = S_CIS if hd == 1 else G_CIS
                    s2_t = st.tile([128, 4], f32, tag=f"s2_{hd}", name=f"s2_{hd}")
                    for m in range(4):
                        lo = m * NCT + cis[0]
                        nc.vector.tensor_reduce(
                            s2_t[:, m : m + 1],
                            sums_t[:, lo : lo + len(cis)],
                            axis=mybir.AxisListType.X,
                            op=mybir.AluOpType.add,
                        )
                    cc_i = dram.tile([128, 4], f32, name=f"cci{hd}")
                    cc_o = dram.tile([128, 4], f32, addr_space="Shared", name=f"cco{hd}")
                    nc.sync.dma_start(out=cc_i[:], in_=s2_t[:])
                    nc.gpsimd.collective_compute(
                        "AllReduce",
                        mybir.AluOpType.add,
                        replica_groups=[list(range(NCORES))],
                        ins=[cc_i.opt()],
                        outs=[cc_o.opt()],
                    )
                    sg_t = st.tile([128, 4], f32, tag=f"sg_{hd}", name=f"sg_{hd}")
                    nc.sync.dma_start(out=sg_t[:], in_=cc_o[:])
                    logs_t = st.tile([128, 4], f32, tag=f"lg_{hd}", name=f"lg_{hd}")
                    nc.scalar.activation(
                        logs_t[:], sg_t[:], mybir.ActivationFunctionType.Ln
                    )
                    if hd == 1:
                        logs_s = logs_t
                    else:
                        logs_g = logs_t

            # ---------------- pass 2: recompute z, subtract log(S) ----------
            for ci in S_CIS + G_CIS:
                head, c0, csz = COLTILES[ci]
                logs = logs_s if head == 1 else logs_g
                for m in range(4):
                    pzt = mm_tile(ci, m, pz)
                    o_t = po.tile([128, 512], f32, tag="po")
                    nc.vector.tensor_scalar(
                        o_t[:, 0:csz],
                        pzt[:, 0:csz],
                        logs[:, m : m + 1],
                        None,
                        op0=mybir.AluOpType.subtract,
                    )
                    dst = outg_d if head == 0 else outs_d
                    lc0 = c0 if head == 0 else c0 - GW
                    nc.sync.dma_start(
                        out=dst[m * 128 : (m + 1) * 128, lc0 : lc0 + csz],
                        in_=o_t[:, 0:csz],
                    )

    nc.compile()
    return nc


def _prep(x, edge_index, edge_type, W_rel, W0, Wg, bg, Ws, bs):
    """Host-side index preprocessing + per-core input shards."""
    x = np.asarray(x, dtype=np.float32)
    ei = np.asarray(edge_index).astype(np.int64)
    et = np.asarray(edge_type).astype(np.int64)
    W_rel = np.asarray(W_rel, dtype=np.float32)
    W0 = np.asarray(W0, dtype=np.float32)
    Wg = np.asarray(Wg, dtype=np.float32)
    bg = np.asarray(bg, dtype=np.float32)
    Ws = np.asarray(Ws, dtype=np.float32)
    bs = np.asarray(bs, dtype=np.float32)

    src, dst = ei[:, 0, :], ei[:, 1, :]
    bidx = np.arange(B)[:, None]
    deg = np.ones((B, R, N), np.float32)
    np.add.at(deg, (bidx, et, dst), np.float32(1.0))
    norm = 1.0 / np.sqrt(deg[bidx, et, src] * deg[bidx, et, dst])  # [B, E]
    C = np.zeros((B, RX, N), np.float32)
    np.add.at(C, (bidx, et, src), np.where(dst == 0, norm, 0.0).astype(np.float32))
    C[:, :R, 0] += 1.0 / deg[:, :, 0]
    C[:, R, 0] += 1.0

    wfull = np.concatenate([W_rel, W0[None]], axis=0)  # [7, 300, 300]
    wcat = np.zeros((128, NKC, D), np.float32)
    for si, (d0, dsz) in enumerate(DCH):
        for r in range(RX):
            wcat[0:dsz, si * RX + r, :] = wfull[r, d0 : d0 + dsz, :]
    wcat = wcat.reshape(128, NKC * D)

    in_maps = []
    for c in range(NCORES):
        sl = slice(c * BPC, (c + 1) * BPC)
        xg = (
            x[sl]
            .reshape(NGRP, 4, N, D)
            .transpose(1, 2, 0, 3)
            .reshape(128, NGRP * D)
            .copy()
        )
        Cc = C[sl].reshape(NGRP, 4, RX, N)
        cb = np.zeros((NGRP, 4, N, RX, 4), np.float32)
        for b in range(4):
            cb[:, b, :, :, b] = Cc[:, b].transpose(0, 2, 1)
        cblk = cb.reshape(NGRP, 128, GRP_COLS).transpose(1, 0, 2).reshape(
            128, NGRP * GRP_COLS
        ).copy()
        wgs = np.concatenate(
            [Wg[:, c * GW : (c + 1) * GW], Ws[:, c * SW : (c + 1) * SW]], axis=1
        )
        bias = np.concatenate([bg[c * GW : (c + 1) * GW], bs[c * SW : (c + 1) * SW]])
        wc = np.zeros((3, 128, GW + SW), np.float32)
        wc[0] = wgs[0:128]
        wc[1] = wgs[128:256]
        wc[2, 0:44] = wgs[256:300]
        wc[2, 44] = bias
        in_maps.append({"xg": xg, "cblk": cblk, "wcat": wcat, "wc": wc})
    return in_maps


def kernel(x, edge_index, edge_type, W_rel, W0, Wg, bg, Ws, bs):
    global LAST_RESULTS
    if "nc" not in _CACHE:
        _CACHE["nc"] = build_nc()
    nc = _CACHE["nc"]
    in_maps = _prep(x, edge_index, edge_type, W_rel, W0, Wg, bg, Ws, bs)
    trace = bool(int(os.environ.get("KERNEL_TRACE", "0")))
    res = bass_utils.run_bass_kernel_spmd(
        nc, in_maps, core_ids=list(range(NCORES)), trace=trace
    )
    LAST_RESULTS = res
    outg = np.concatenate([res.results[c]["outg"] for c in range(NCORES)], axis=1)
    outs = np.concatenate([res.results[c]["outs"] for c in range(NCORES)], axis=1)
    return outg, outs


# revision 10
# speedup vs baseline: 1.4480x; 1.1344x over previous
"""Trainium2 Bass kernel for CompositeRGCN (8 NeuronCores).

Math (per sample): only node 0's output is consumed downstream, so the whole
R-GCN layer collapses to a row-0-coefficient form.  With

  deg[r, i] = 1 + #edges(et=r, dst=i)
  norm_e    = rsqrt(deg[et_e, src_e] * deg[et_e, dst_e])
  C[r, n]   = sum_{e: dst_e=0, et_e=r, src_e=n} norm_e + (n==0)/deg[r, 0]
  C[R, 0]   = 1                      (self-connection x @ W0 as an extra "relation")

the node-0 hidden state is
  h0 = leaky_relu( sum_{r,n} C[r, n] * (x[n] @ Wcat[r]), 0.1 ),  Wcat = [W_rel; W0]
and the outputs are log_softmax(h0 @ Wg + bg), log_softmax(h0 @ Ws + bs).

Distribution: data-parallel over batch for the graph part (64 samples/core),
AllGather of h0, then tensor-parallel over the 50k/25k output columns with an
AllReduce of the per-row exp-sums for log_softmax.

Kernel structure (v2): all output-projection weights are SBUF-resident and
prefetched during the collectives' entry barrier; softmax pass 2 recomputes
the logits from the resident weights (PE is idle there) and subtracts log(S)
on the vector engine, so logits never round-trip through a low-precision
store.  The S head's stats AllReduce is issued early so its pass 2 overlaps
the G head's pass 1.

C (pure index preprocessing, ~0.4 MB) is built on host; all FLOPs (the
C@x contraction, the 2100x300 projection, the 512x300x75000 output matmuls,
exp/log-softmax) run on the NeuronCores.
"""

import os
import sys

sys.path.insert(0, "/opt/trn_rl_repo")

import numpy as np

import concourse.bass as bass  # noqa: F401
import concourse.mybir as mybir
import concourse.tile as tile
from concourse import bacc, bass_utils

# Problem shapes (hardcoded per contract).
B, N, D, R, E = 512, 32, 300, 6, 256
NG, NS = 50000, 25000
NCORES = 8
BPC = B // NCORES           # 64 samples per core
NGRP = BPC // 4             # 16 groups of 4 samples (4*32 = 128 = K)
RX = R + 1                  # 7 "relations" including W0
GRP_COLS = RX * 4           # 28 columns per group in the block-diag C
KH = D + 1                  # 301 contraction rows for the head matmul (bias row)
GW, SW = NG // NCORES, NS // NCORES   # per-core head widths: 6250 / 3125

DCH = [(0, 128), (128, 128), (256, 44)]
JCH = [(0, 128), (128, 128), (256, 45)]   # rows of [h; 1] / [W; bias]
NKC = len(DCH) * RX                       # 21 K-chunks of the 2100-dim contraction


def _col_tiles():
    tiles = []
    for head, base, w in ((0, 0, GW), (1, GW, SW)):
        st = 0
        while st < w:
            csz = min(512, w - st)
            tiles.append((head, base + st, csz))
            st += csz
    return tiles


COLTILES = _col_tiles()          # 13 G tiles + 7 S tiles
NGT = sum(1 for t in COLTILES if t[0] == 0)
NCT = len(COLTILES)
S_CIS = [ci for ci, t in enumerate(COLTILES) if t[0] == 1]
G_CIS = [ci for ci, t in enumerate(COLTILES) if t[0] == 0]

_CACHE = {}
LAST_RESULTS = None


def build_nc():
    nc = bacc.Bacc("TRN2", target_bir_lowering=False, debug=False, num_devices=NCORES)
    f32, f32r, bf16 = mybir.dt.float32, mybir.dt.float32r, mybir.dt.bfloat16

    xg_d = nc.dram_tensor("xg", [128, NGRP * D], f32, kind="ExternalInput")
    cblk_d = nc.dram_tensor("cblk", [128, NGRP * GRP_COLS], f32, kind="ExternalInput")
    wcat_d = nc.dram_tensor("wcat", [128, NKC * D], f32, kind="ExternalInput")
    # output-projection weights, pre-chunked; tagged float32r so the PE can
    # stream them at full rate with no on-chip cast (f32r = hw-rounded f32)
    wc_d = nc.dram_tensor("wc", [3, 128, GW + SW], f32r, kind="ExternalInput")
    outg_d = nc.dram_tensor("outg", [B, GW], f32, kind="ExternalOutput")
    outs_d = nc.dram_tensor("outs", [B, SW], f32, kind="ExternalOutput")

    with tile.TileContext(nc) as tc:
        with (
            tc.tile_pool(name="io", bufs=1) as io,
            tc.tile_pool(name="gn", bufs=1) as gn,
            tc.tile_pool(name="wres", bufs=1) as wres,
            tc.tile_pool(name="ze", bufs=1) as ze,
            tc.tile_pool(name="st", bufs=1) as st,
            tc.tile_pool(name="po", bufs=3) as po,
            tc.tile_pool(name="pg1", bufs=3, space="PSUM") as pg1,
            tc.tile_pool(name="pg2", bufs=2, space="PSUM") as pg2,
            tc.tile_pool(name="pz", bufs=3, space="PSUM") as pz,
            tc.tile_pool(name="dram", bufs=1, space="DRAM") as dram,
        ):
            # ---------------- Phase G: per-sample graph contraction ----------
            # (input DMAs first so the graph phase starts immediately; the
            # weight prefetch below fills the collectives' entry barrier)
            xg_t = io.tile([128, NGRP * D], f32)
            cblk_t = io.tile([128, NGRP * GRP_COLS], f32)
            wcat_t = io.tile([128, NKC * D], f32)
            nc.sync.dma_start(out=xg_t[:], in_=xg_d.ap())
            nc.sync.dma_start(out=cblk_t[:], in_=cblk_d.ap())
            nc.sync.dma_start(out=wcat_t[:], in_=wcat_d.ap())

            # -------- streamed output-projection weights (deep prefetch) ----
            wtiles = {}
            for ci in S_CIS + G_CIS:  # prefetch in consumption order
                head, c0, csz = COLTILES[ci]
                for j, (j0, jsz) in enumerate(JCH):
                    w_t = wres.tile([128, 512], f32r, tag="w", bufs=18)
                    nc.sync.dma_start(
                        out=w_t[0:jsz, 0:csz], in_=wc_d[j, 0:jsz, c0 : c0 + csz]
                    )
                    wtiles[(ci, j)] = w_t

            ut_t = gn.tile([128, NKC * BPC], f32)
            ut_v = ut_t.rearrange("p (k e) -> p k e", k=NKC)
            for g in range(NGRP):
                for si, (d0, dsz) in enumerate(DCH):
                    pmm = pg1.tile([128, GRP_COLS], f32, tag="g1")
                    nc.tensor.matmul(
                        pmm[0:dsz, :],
                        lhsT=xg_t[:, g * D + d0 : g * D + d0 + dsz],
                        rhs=cblk_t[:, g * GRP_COLS : (g + 1) * GRP_COLS],
                        start=True,
                        stop=True,
                    )
                    src = pmm[0:dsz, :].rearrange("p (r e) -> p r e", r=RX)
                    dst = ut_v[0:dsz, si * RX : (si + 1) * RX, g * 4 : g * 4 + 4]
                    eng = nc.vector if (g % 2 == 0) else nc.scalar
                    if eng is nc.vector:
                        nc.vector.tensor_copy(dst, src)
                    else:
                        nc.scalar.copy(dst, src)

            h0_t = gn.tile([128, 3 * BPC], f32)
            nc.vector.memset(h0_t[:], 0.0)
            for mi, (m0, msz) in enumerate(DCH):
                pacc = pg2.tile([128, BPC], f32, tag="g2")
                for k in range(NKC):
                    ksz = DCH[k // RX][1]
                    nc.tensor.matmul(
                        pacc[0:msz, :],
                        lhsT=wcat_t[0:ksz, k * D + m0 : k * D + m0 + msz],
                        rhs=ut_v[0:ksz, k, :],
                        start=(k == 0),
                        stop=(k == NKC - 1),
                    )
                lk_t = gn.tile([128, BPC], f32, tag="lk", bufs=2)
                nc.scalar.mul(lk_t[0:msz, :], pacc[0:msz, :], 0.1)
                nc.vector.tensor_max(
                    h0_t[0:msz, mi * BPC : (mi + 1) * BPC],
                    pacc[0:msz, :],
                    lk_t[0:msz, :],
                )
            # bias row (K index 300) = 1.0 at chunk 2, partition 44 (DMA: engines
            # can't start at partition 44)
            ones_t = gn.tile([1, BPC], f32)
            nc.vector.memset(ones_t[:], 1.0)
            nc.gpsimd.dma_start(out=h0_t[44:45, 2 * BPC : 3 * BPC], in_=ones_t[0:1, :])

            # ---------------- AllGather h0 across the 8 cores ----------------
            cc_h0i = dram.tile([128, 3 * BPC], f32)
            cc_h0o = dram.tile([NCORES, 128, 3 * BPC], f32, addr_space="Shared")
            nc.gpsimd.dma_start(out=cc_h0i[:], in_=h0_t[:])
            nc.gpsimd.collective_compute(
                "AllGather",
                mybir.AluOpType.bypass,
                replica_groups=[list(range(NCORES))],
                ins=[cc_h0i.opt()],
                outs=[cc_h0o.opt()],
            )
            hf_t = st.tile([128, 3 * B], f32)
            hr_t = st.tile([128, 3 * B], f32r)
            for j, (j0, jsz) in enumerate(JCH):
                nc.gpsimd.dma_start(
                    out=hf_t[0:jsz, j * B : (j + 1) * B].rearrange(
                        "p (c e) -> p c e", c=NCORES
                    ),
                    in_=cc_h0o[:, 0:jsz, j * BPC : (j + 1) * BPC].rearrange(
                        "c p e -> p c e"
                    ),
                )
                nc.vector.tensor_copy(
                    hr_t[0:jsz, j * B : (j + 1) * B], hf_t[0:jsz, j * B : (j + 1) * B]
                )

            # PE clock warmup: the HAM un-throttles (1.2 -> 2.4 GHz) only after
            # ~3.4us of dense activity; a short back-to-back f32r burst right
            # after the gather trips it before the real matmul stream begins.
            for i in range(24):
                pw = pz.tile([128, 512], f32, tag="z")
                nc.tensor.matmul(
                    pw[:], lhsT=hr_t[:, 0:128], rhs=hr_t[:, 0:512],
                    start=True, stop=True,
                )

            # ---------------- pass 1: matmul + exp row-sums ------------------
            sums_t = st.tile([128, 4 * NCT], f32)

            def mm_tile(ci, m, ppool):
                head, c0, csz = COLTILES[ci]
                use_r = csz >= 256
                pzt = ppool.tile([128, 512], f32, tag="z")
                for j, (j0, jsz) in enumerate(JCH):
                    rhs = wtiles[(ci, j)][0:jsz, 0:csz]
                    lhs = (hr_t if use_r else hf_t)[
                        0:jsz, j * B + m * 128 : j * B + (m + 1) * 128
                    ]
                    if not use_r:
                        rhs = rhs.bitcast(mybir.dt.float32)
                    nc.tensor.matmul(
                        pzt[:, 0:csz], lhsT=lhs, rhs=rhs,
                        start=(j == 0), stop=(j == 2),
                    )
                return pzt

            z_tiles = {}
            for ci in S_CIS + G_CIS:
                head, c0, csz = COLTILES[ci]
                for m in range(4):
                    pzt = mm_tile(ci, m, pz)
                    e_t = ze.tile(
                        [128, csz], mybir.dt.bfloat16,
                        tag=f"z{ci}_{m}", name=f"z{ci}_{m}",
                    )
                    nc.scalar.activation(
                        e_t[:],
                        pzt[:, 0:csz],
                        mybir.ActivationFunctionType.Exp,
                        accum_out=sums_t[:, m * NCT + ci : m * NCT + ci + 1],
                    )
                    z_tiles[(ci, m)] = e_t

                if ci == S_CIS[-1] or ci == G_CIS[-1]:
                    # this head's pass-1 done: reduce + AllReduce its sums
                    hd = COLTILES[ci][0]
                    cis = S_CIS if hd == 1 else G_CIS
                    s2_t = st.tile([128, 4], f32, tag=f"s2_{hd}", name=f"s2_{hd}")
                    for m in range(4):
                        lo = m * NCT + cis[0]
                        nc.vector.tensor_reduce(
                            s2_t[:, m : m + 1],
                            sums_t[:, lo : lo + len(cis)],
                            axis=mybir.AxisListType.X,
                            op=mybir.AluOpType.add,
                        )
                    cc_i = dram.tile([128, 4], f32, name=f"cci{hd}")
                    cc_o = dram.tile([128, 4], f32, addr_space="Shared", name=f"cco{hd}")
                    nc.gpsimd.dma_start(out=cc_i[:], in_=s2_t[:])
                    nc.gpsimd.collective_compute(
                        "AllReduce",
                        mybir.AluOpType.add,
                        replica_groups=[list(range(NCORES))],
                        ins=[cc_i.opt()],
                        outs=[cc_o.opt()],
                    )
                    sg_t = st.tile([128, 4], f32, tag=f"sg_{hd}", name=f"sg_{hd}")
                    nc.gpsimd.dma_start(out=sg_t[:], in_=cc_o[:])
                    rcp_t = st.tile([128, 4], f32, tag=f"rc_{hd}", name=f"rc_{hd}")
                    nc.vector.reciprocal(rcp_t[:], sg_t[:])
                    if hd == 1:
                        rcp_s = rcp_t
                    else:
                        rcp_g = rcp_t

            # ------- pass 2: out = ln(exp(z) / S) from the stored exps ------
            for ci in S_CIS + G_CIS:
                head, c0, csz = COLTILES[ci]
                rcp = rcp_s if head == 1 else rcp_g
                for m in range(4):
                    o_t = po.tile([128, 512], f32, tag="po")
                    nc.scalar.activation(
                        o_t[:, 0:csz],
                        z_tiles[(ci, m)][:],
                        mybir.ActivationFunctionType.Ln,
                        scale=rcp[:, m : m + 1],
                    )
                    dst = outg_d if head == 0 else outs_d
                    lc0 = c0 if head == 0 else c0 - GW
                    nc.sync.dma_start(
                        out=dst[m * 128 : (m + 1) * 128, lc0 : lc0 + csz],
                        in_=o_t[:, 0:csz],
                    )

    nc.compile()
    return nc


def _prep(x, edge_index, edge_type, W_rel, W0, Wg, bg, Ws, bs):
    """Host-side index preprocessing + per-core input shards."""
    x = np.asarray(x, dtype=np.float32)
    ei = np.asarray(edge_index).astype(np.int64)
    et = np.asarray(edge_type).astype(np.int64)
    W_rel = np.asarray(W_rel, dtype=np.float32)
    W0 = np.asarray(W0, dtype=np.float32)
    Wg = np.asarray(Wg, dtype=np.float32)
    bg = np.asarray(bg, dtype=np.float32)
    Ws = np.asarray(Ws, dtype=np.float32)
    bs = np.asarray(bs, dtype=np.float32)

    src, dst = ei[:, 0, :], ei[:, 1, :]
    bidx = np.arange(B)[:, None]
    deg = np.ones((B, R, N), np.float32)
    np.add.at(deg, (bidx, et, dst), np.float32(1.0))
    norm = 1.0 / np.sqrt(deg[bidx, et, src] * deg[bidx, et, dst])  # [B, E]
    C = np.zeros((B, RX, N), np.float32)
    np.add.at(C, (bidx, et, src), np.where(dst == 0, norm, 0.0).astype(np.float32))
    C[:, :R, 0] += 1.0 / deg[:, :, 0]
    C[:, R, 0] += 1.0

    wfull = np.concatenate([W_rel, W0[None]], axis=0)  # [7, 300, 300]
    wcat = np.zeros((128, NKC, D), np.float32)
    for si, (d0, dsz) in enumerate(DCH):
        for r in range(RX):
            wcat[0:dsz, si * RX + r, :] = wfull[r, d0 : d0 + dsz, :]
    wcat = wcat.reshape(128, NKC * D)

    in_maps = []
    for c in range(NCORES):
        sl = slice(c * BPC, (c + 1) * BPC)
        xg = (
            x[sl]
            .reshape(NGRP, 4, N, D)
            .transpose(1, 2, 0, 3)
            .reshape(128, NGRP * D)
            .copy()
        )
        Cc = C[sl].reshape(NGRP, 4, RX, N)
        cb = np.zeros((NGRP, 4, N, RX, 4), np.float32)
        for b in range(4):
            cb[:, b, :, :, b] = Cc[:, b].transpose(0, 2, 1)
        cblk = cb.reshape(NGRP, 128, GRP_COLS).transpose(1, 0, 2).reshape(
            128, NGRP * GRP_COLS
        ).copy()
        wgs = np.concatenate(
            [Wg[:, c * GW : (c + 1) * GW], Ws[:, c * SW : (c + 1) * SW]], axis=1
        )
        bias = np.concatenate([bg[c * GW : (c + 1) * GW], bs[c * SW : (c + 1) * SW]])
        wc = np.zeros((3, 128, GW + SW), np.float32)
        wc[0] = wgs[0:128]
        wc[1] = wgs[128:256]
        wc[2, 0:44] = wgs[256:300]
        wc[2, 44] = bias
        in_maps.append({"xg": xg, "cblk": cblk, "wcat": wcat, "wc": wc})
    return in_maps


def kernel(x, edge_index, edge_type, W_rel, W0, Wg, bg, Ws, bs):
    global LAST_RESULTS
    if "nc" not in _CACHE:
        _CACHE["nc"] = build_nc()
    nc = _CACHE["nc"]
    in_maps = _prep(x, edge_index, edge_type, W_rel, W0, Wg, bg, Ws, bs)
    trace = bool(int(os.environ.get("KERNEL_TRACE", "0")))
    res = bass_utils.run_bass_kernel_spmd(
        nc, in_maps, core_ids=list(range(NCORES)), trace=trace
    )
    LAST_RESULTS = res
    outg = np.concatenate([res.results[c]["outg"] for c in range(NCORES)], axis=1)
    outs = np.concatenate([res.results[c]["outs"] for c in range(NCORES)], axis=1)
    return outg, outs
